# revision 8
# baseline (speedup 1.0000x reference)
# Trainium2 Bass kernel for NeighborhoodAugmenter (retrieval_knn).
#
# reference semantics:
#   h_norm = latent / ||latent||            (rows)
#   sim    = h_norm @ h_norm.T;  diag -> -9e15
#   top3   = top_k(sim, 3) indices; pick rand_idx-th per row
#   out    = where(unif < 0.8, x, x[neighbor])
#
# Strategy (8 cores, batch-sharded, x replicated so the neighbor gather is
# local):
#   Per core (S=1024 rows): v = latent_shard @ hnT with hnT the
#   column-normalized transposed latent (host-precomputed; row scaling does
#   not change per-row order, and the self column is always rank-0, which
#   replaces the diagonal mask).  Top-8 per row via DVE max/max_index,
#   neighbor = idx8[:, 1 + rand_idx].  Indirect-DMA gather of neighbor rows
#   of the full x, then out = where(unif < 0.8, x_shard, gathered).
import numpy as np

B, G, D = 8192, 20000, 64
N_CORES = 8
S = B // N_CORES          # rows per core
TP = 128                  # rows per row-tile
NT = S // TP              # row-tiles per core
W = 5000                  # load chunk width (20 KB per row descriptor)
WG = 10000                # gather/store chunk width (40 KB per row descriptor)
MIX = 0.8

_PROG = None


def build_program(nc, b, g, s, w, wg):
    import concourse.bass as bass
    import concourse.tile as tile
    from concourse import mybir
    from concourse.bass import ds, ts

    f32 = mybir.dt.float32
    i32 = mybir.dt.int32
    u32 = mybir.dt.uint32
    i8 = mybir.dt.int8
    AX = mybir.AxisListType
    OP = mybir.AluOpType

    tp = 128
    nt = s // tp
    nj = b // 512             # 512-wide matmul column chunks
    nw = g // w               # load chunks per row-tile
    nwg = g // wg             # gather/store chunks per row-tile
    hpg = wg // w             # load chunks per gather chunk

    xf = nc.dram_tensor("xf", [b, g], f32, kind="ExternalInput").ap()
    hnt = nc.dram_tensor("hnt", [D, b], f32, kind="ExternalInput").ap()
    latTs = nc.dram_tensor("latTs", [D, s], f32, kind="ExternalInput").ap()
    xs = nc.dram_tensor("xs", [s, g], f32, kind="ExternalInput").ap()
    unif = nc.dram_tensor("unif", [s, g], f32, kind="ExternalInput").ap()
    rnd = nc.dram_tensor("rnd", [tp, nt], i32, kind="ExternalInput").ap()
    out = nc.dram_tensor("out", [s, g], f32, kind="ExternalOutput").ap()

    with tile.TileContext(nc) as tc:
        with (
            tc.tile_pool(name="const", bufs=1) as cpool,
            tc.tile_pool(name="simp", bufs=1) as simpool,
            tc.tile_pool(name="gat", bufs=1) as gpool,
            tc.tile_pool(name="xch", bufs=2) as xpool,
            tc.tile_pool(name="uch", bufs=2) as upool,
            tc.tile_pool(name="msk", bufs=2) as mpool,
            tc.tile_pool(name="small", bufs=2) as spool,
            tc.tile_pool(name="mm", bufs=4, space="PSUM") as mmpool,
        ):
            iota8 = cpool.tile([tp, 8], i32)
            nc.gpsimd.iota(iota8[:], pattern=[[1, 8]], base=-1, channel_multiplier=0)
            rnd_sb = cpool.tile([tp, nt], i32)
            nc.sync.dma_start(rnd_sb[:], rnd)
            latTs_sb = cpool.tile([D, s], f32)
            nc.sync.dma_start(latTs_sb[:], latTs)
            hnT = cpool.tile([D, b], f32)
            nc.sync.dma_start(hnT[:], hnt)

            for t in range(nt):
                # x/unif chunk loads do not depend on the neighbor index:
                # emit them first so they fill the pre-gather window.
                xcs, mks = [], []
                for ci in range(nw):
                    c0 = ci * w
                    xc = xpool.tile([tp, w], f32, tag="xc")
                    nc.sync.dma_start(xc[:], xs[ds(t * tp, tp), ds(c0, w)])
                    uc = upool.tile([tp, w], f32, tag="uc")
                    nc.sync.dma_start(uc[:], unif[ds(t * tp, tp), ds(c0, w)])
                    mk = mpool.tile([tp, w], i8, tag="mk")
                    nc.vector.tensor_scalar(
                        out=mk[:], in0=uc[:], scalar1=MIX, scalar2=None, op0=OP.is_lt
                    )
                    xcs.append(xc)
                    mks.append(mk)

                sim = simpool.tile([tp, b], f32, tag="sim")
                for j in range(nj):
                    ps = mmpool.tile([tp, 512], f32, tag="mm")
                    nc.tensor.matmul(
                        ps[:],
                        lhsT=latTs_sb[:, ts(t, tp)],
                        rhs=hnT[:, ts(j, 512)],
                        start=True,
                        stop=True,
                    )
                    nc.scalar.copy(out=sim[:, ts(j, 512)], in_=ps[:])
                mx = spool.tile([tp, 8], f32, tag="mx")
                nc.vector.max(out=mx[:], in_=sim[:])
                mi = spool.tile([tp, 8], u32, tag="mi")
                nc.vector.max_index(out=mi[:], in_max=mx[:], in_values=sim[:])
                eq = spool.tile([tp, 8], i32, tag="eq")
                nc.vector.tensor_tensor(
                    out=eq[:],
                    in0=iota8[:],
                    in1=rnd_sb[:, t : t + 1].to_broadcast([tp, 8]),
                    op=OP.is_equal,
                )
                pr = spool.tile([tp, 8], i32, tag="pr")
                nc.vector.tensor_tensor(
                    out=pr[:], in0=eq[:], in1=mi[:].bitcast(i32), op=OP.mult
                )
                nbr = spool.tile([tp, 1], i32, tag="nbr")
                with nc.allow_low_precision(reason="int32 index select, exact"):
                    nc.vector.reduce_sum(out=nbr[:], in_=pr[:], axis=AX.X)

                for gi in range(nwg):
                    gt = gpool.tile([tp, wg], tag="gt", dtype=f32)
                    nc.gpsimd.indirect_dma_start(
                        out=gt[:],
                        out_offset=None,
                        in_=xf,
                        in_offset=bass.IndirectOffsetOnAxis(ap=nbr[:, :1], axis=0),
                        element_offset=gi * wg,
                    )
                    for h in range(hpg):
                        ci = gi * hpg + h
                        nc.vector.copy_predicated(
                            out=gt[:, ds(h * w, w)], mask=mks[ci][:], data=xcs[ci][:]
                        )
                    nc.sync.dma_start(out[ds(t * tp, tp), ds(gi * wg, wg)], gt[:])
    return nc


def _get_prog():
    global _PROG
    if _PROG is None:
        from concourse import bacc

        nc = bacc.Bacc(
            "TRN2", target_bir_lowering=False, debug=False, num_devices=N_CORES
        )
        build_program(nc, B, G, S, W, WG)
        nc.compile()
        _PROG = nc
    return _PROG


def make_hnt(latent):
    lat64 = latent.astype(np.float64)
    hn = lat64 / np.sqrt((lat64 * lat64).sum(axis=1, keepdims=True))
    return np.ascontiguousarray(hn.T.astype(np.float32))


def make_in_maps(x, latent, rand_idx, unif):
    x = np.ascontiguousarray(np.asarray(x, dtype=np.float32))
    latent = np.ascontiguousarray(np.asarray(latent, dtype=np.float32))
    rand_idx = np.asarray(rand_idx, dtype=np.int32)
    unif = np.ascontiguousarray(np.asarray(unif, dtype=np.float32))
    hnt = make_hnt(latent)
    in_maps = []
    for c in range(N_CORES):
        r0 = c * S
        in_maps.append(
            {
                "xf": x,
                "hnt": hnt,
                "latTs": np.ascontiguousarray(latent[r0 : r0 + S].T),
                "xs": x[r0 : r0 + S],
                "unif": unif[r0 : r0 + S],
                "rnd": np.ascontiguousarray(
                    rand_idx[r0 : r0 + S].reshape(NT, TP).T
                ),
            }
        )
    return in_maps


def kernel(x, latent, rand_idx, unif):
    from concourse.bass_utils import run_bass_kernel_spmd

    nc = _get_prog()
    in_maps = make_in_maps(x, latent, rand_idx, unif)
    res = run_bass_kernel_spmd(nc, in_maps, core_ids=list(range(N_CORES)))
    return np.concatenate([res.results[c]["out"] for c in range(N_CORES)], axis=0)
